# revision 10
# baseline (speedup 1.0000x reference)
"""Trainium2 Bass kernel for nn_AttentionLayer (B=64, S=512, F=256), 8 cores.

Reference computation (per batch b):
    scores = x1 @ Wq + x2 @ Wk           # [S, S]
    a = softmax(tanh(scores), axis=-1)   # softmax over u
    a2 = a @ Wv                          # [S, S]
    out = a2 * x1                        # elementwise
    out = out * rsqrt(max(sum_s out^2, eps))   # l2-normalize over axis s

Strategy: pure data parallelism — 8 batches per core, weights replicated.
All tensors live in a TRANSPOSED layout ([t-or-u partitions, s free]) and in
fp16 end-to-end: same PE matmul rate as fp32r/bf16, half the HBM traffic of
fp32, and 2-4x DVE throughput vs fp32.

Engine budget per batch (errata-adjusted cost model, FD=512):
    PE   41 matmul passes (24 scores + 1 rowsum + 16 attn-out)   ~8.9us
    ACT  tanh+exp (in-place in PSUM) + 4x Square-with-accum      ~7.6us
    DVE  esum adds, recip, xb=x1*r, p=y*xb (PSUM drain), Newton
         rsqrt, final scales                                      ~7.2us
    Pool partition_broadcast + out-DMA issue                      ~2us

Key structural choices:
  * Softmax normalization DEFERRED past the second matmul:
    (exp(z)/D) @ Wv == (exp(z) @ Wv) / D (D varies only along the free
    axis s), so the TensorEngine never waits on the denominator chain.
    The 1/D scale is folded into x1 (xb = x1*r, off the critical path),
    leaving ONE fp32 PSUM-read op per output tile: p = y * xb.
  * The denominator's partition reduction: 4 exp u-tiles pre-summed on the
    DVE (fp16 adds), then a single ones-vector matmul pass on the PE.
  * l2-norm rsqrt on the DVE via Quake-style bit-trick seed + one Newton
    step (max rel err 0.18%) — the ACT engine only ever uses the
    exp_and_others table (tanh/exp/square): zero table reloads.
  * 1-batch software-pipeline skew; PE order [rowsum(b-1)][A(b)][C(b-1)].
"""

import sys

sys.path.insert(0, "/opt/trn_rl_repo")

import numpy as np

import concourse.bass as bass
import concourse.tile as tile
from concourse import bacc, mybir
from concourse.bass_utils import run_bass_kernel_spmd

B, S, F = 64, 512, 256
N_CORES = 8
BPC = B // N_CORES  # batches per core
P = 128
KT1 = S // P  # 4 k-tiles over t (x1/Wq contraction)
KT2 = F // P  # 2 k-tiles over f (x2/Wk contraction)
NT = S // P  # 4 m-tiles over u (stage A) / t (stage C)

F32 = mybir.dt.float32
F16 = mybir.dt.float16
I32 = mybir.dt.int32
AF = mybir.ActivationFunctionType
ALU = mybir.AluOpType

MAGIC = 0x5F375A86  # rsqrt seed constant

last_results = None  # test harness introspection


def build_nc(bpc=BPC):
    nc = bacc.Bacc(
        "TRN2", target_bir_lowering=False, debug=False, num_devices=N_CORES
    )
    x1t = nc.declare_dram_parameter("x1t", [bpc, S, S], F16, isOutput=False)
    x2t = nc.declare_dram_parameter("x2t", [bpc, F, S], F16, isOutput=False)
    wq = nc.declare_dram_parameter("wq", [S, S], F16, isOutput=False)
    wk = nc.declare_dram_parameter("wk", [F, S], F16, isOutput=False)
    wv = nc.declare_dram_parameter("wv", [S, S], F16, isOutput=False)
    out = nc.declare_dram_parameter("out", [bpc, S, S], F16, isOutput=True)

    x1t_r = x1t.ap().rearrange("b (a p) s -> b a p s", p=P)
    x2t_r = x2t.ap().rearrange("b (a p) s -> b a p s", p=P)
    out_r = out.ap().rearrange("b (a p) s -> b a p s", p=P)
    wq_r = wq.ap().rearrange("(a p) u -> a p u", p=P)
    wk_r = wk.ap().rearrange("(a p) u -> a p u", p=P)
    wv_r = wv.ap().rearrange("(a p) t -> a p t", p=P)

    with tile.TileContext(nc) as tc:
        with (
            tc.tile_pool(name="singles", bufs=1) as singles,
            tc.tile_pool(name="xin", bufs=1) as xin,
            tc.tile_pool(name="work", bufs=1) as work,
            tc.tile_pool(name="epil", bufs=1) as epil,
            tc.tile_pool(name="small", bufs=2) as small,
            tc.tile_pool(name="psA", bufs=2, space="PSUM") as psA,
            tc.tile_pool(name="psY", bufs=3, space="PSUM") as psY,
            tc.tile_pool(name="psR", bufs=1, space="PSUM") as psR,
        ):
            # ---- startup DMAs in first-use order, striped across the three
            # DMA-capable queues (scalar starts earliest on this runtime).
            wq_sb = [
                singles.tile([P, S], F16, tag=f"wq{kt}", name=f"wq{kt}")
                for kt in range(KT1)
            ]
            wk_sb = [
                singles.tile([P, S], F16, tag=f"wk{kt}", name=f"wk{kt}")
                for kt in range(KT2)
            ]
            wv_sb = [
                singles.tile([P, S], F16, tag=f"wv{ut}", name=f"wv{ut}")
                for ut in range(NT)
            ]
            x1_first = xin.tile([P, KT1, S], F16, tag="x1", bufs=4)
            x2_first = xin.tile([P, KT2, S], F16, tag="x2", bufs=3)
            nc.scalar.dma_start(out=wq_sb[0], in_=wq_r[0])
            nc.sync.dma_start(out=x1_first[:, 0, :], in_=x1t_r[0, 0])
            nc.gpsimd.dma_start(out=wq_sb[1], in_=wq_r[1])
            nc.scalar.dma_start(out=x1_first[:, 1, :], in_=x1t_r[0, 1])
            nc.sync.dma_start(out=wq_sb[2], in_=wq_r[2])
            nc.gpsimd.dma_start(out=x1_first[:, 2, :], in_=x1t_r[0, 2])
            nc.scalar.dma_start(out=wq_sb[3], in_=wq_r[3])
            nc.sync.dma_start(out=x1_first[:, 3, :], in_=x1t_r[0, 3])
            nc.gpsimd.dma_start(out=wk_sb[0], in_=wk_r[0])
            nc.scalar.dma_start(out=x2_first[:, 0, :], in_=x2t_r[0, 0])
            nc.sync.dma_start(out=wk_sb[1], in_=wk_r[1])
            nc.gpsimd.dma_start(out=x2_first[:, 1, :], in_=x2t_r[0, 1])
            for ut in range(NT):
                [nc.scalar, nc.sync, nc.gpsimd, nc.scalar][ut].dma_start(
                    out=wv_sb[ut], in_=wv_r[ut]
                )

            ones_col = singles.tile([P, 1], F16)
            nc.vector.memset(ones_col, 1.0)
            magic_t = singles.tile([P, NT], I32)
            nc.vector.memset(magic_t, MAGIC)
            shift1 = singles.tile([P, 1], I32)
            nc.vector.memset(shift1, 1)

            def dma_x1(b):
                t = xin.tile([P, KT1, S], F16, tag="x1", bufs=4)
                nc.sync.dma_start(out=t, in_=x1t_r[b].rearrange("a p s -> p a s"))
                return t

            def dma_x2(b):
                t = xin.tile([P, KT2, S], F16, tag="x2", bufs=3)
                nc.gpsimd.dma_start(
                    out=t, in_=x2t_r[b].rearrange("a p s -> p a s")
                )
                return t

            def stage_a(x1_sb, x2_sb):
                """scores matmuls (u-tile pairs sharing a 2-bank PSUM tile),
                tanh in-place in PSUM + exp on ACT, first exp pre-sum on DVE.
                Returns (expz, esum); esum still needs adds 2 and 3 (emitted
                later via finish_esum so mid-pipeline DVE work isn't blocked
                behind the exp of this batch)."""
                expz = work.tile([P, NT, S], F16, tag="expz", bufs=3)
                esum = work.tile([P, S], F16, tag="esum", bufs=3)
                for half in range(NT // 2):
                    sc = psA.tile([P, 2, S], F32, tag="scores")
                    for j in range(2):
                        ut = half * 2 + j
                        for kt in range(KT1):
                            nc.tensor.matmul(
                                sc[:, j, :],
                                wq_sb[kt][:, ut * P : (ut + 1) * P],
                                x1_sb[:, kt, :],
                                start=(kt == 0),
                                stop=False,
                            )
                        for kt in range(KT2):
                            nc.tensor.matmul(
                                sc[:, j, :],
                                wk_sb[kt][:, ut * P : (ut + 1) * P],
                                x2_sb[:, kt, :],
                                start=False,
                                stop=(kt == KT2 - 1),
                            )
                    nc.scalar.activation(out=sc, in_=sc, func=AF.Tanh)
                    nc.scalar.activation(
                        out=expz[:, half * 2 : half * 2 + 2, :],
                        in_=sc,
                        func=AF.Exp,
                    )
                return expz, esum

            def finish_esum(expz, esum):
                e01 = work.tile([P, 2, S], F16, tag="e01", bufs=2)
                nc.vector.tensor_tensor(
                    out=e01, in0=expz[:, 0:2, :], in1=expz[:, 2:4, :], op=ALU.add
                )
                nc.vector.tensor_tensor(
                    out=esum, in0=e01[:, 0, :], in1=e01[:, 1, :], op=ALU.add
                )

            def stage_r(esum, x1_sb):
                """deferred softmax denominator: one ones-matmul pass, fast
                reciprocal, fp16 cast, partition broadcast, then fold into
                x1: xb = x1 * (1/D) — all off the PE critical path."""
                rs = psR.tile([1, S], F32, tag="rowsum")
                nc.tensor.matmul(rs, ones_col, esum, start=True, stop=True)
                recip_f = small.tile([1, S], F32, tag="recipf")
                nc.vector.reciprocal_approx_fast(out=recip_f, in_=rs)
                recip_h = small.tile([1, S], F16, tag="reciph")
                nc.vector.tensor_copy(out=recip_h, in_=recip_f)
                bc4 = small.tile([P, NT, S], F16, tag="bc4")
                for tt in range(NT):
                    nc.gpsimd.partition_broadcast(bc4[:, tt, :], recip_h)
                xb = epil.tile([P, NT, S], F16, tag="xb", bufs=2)
                nc.vector.tensor_tensor(out=xb, in0=x1_sb, in1=bc4, op=ALU.mult)
                return xb

            def newton_rsqrt(ss):
                """rsq ~ 1/sqrt(ss) on DVE via bit-trick seed + 1 Newton;
                result in fp16 so the final scales hit the 4x DVE mode."""
                ihalf = small.tile([P, NT], I32, tag="ihalf", bufs=2)
                nc.vector.tensor_scalar(
                    out=ihalf, in0=ss.bitcast(I32), scalar1=shift1, scalar2=None,
                    op0=ALU.logical_shift_right,
                )
                y0i = small.tile([P, NT], I32, tag="y0i", bufs=2)
                nc.vector.tensor_tensor(
                    out=y0i, in0=magic_t, in1=ihalf, op=ALU.subtract
                )
                y0 = y0i.bitcast(F32)
                t0 = small.tile([P, NT], F32, tag="nt0", bufs=2)
                nc.vector.tensor_tensor(out=t0, in0=y0, in1=y0, op=ALU.mult)
                t1 = small.tile([P, NT], F32, tag="nt1", bufs=2)
                nc.vector.scalar_tensor_tensor(
                    out=t1, in0=ss, scalar=-0.5, in1=t0, op0=ALU.mult, op1=ALU.mult
                )
                rsq = small.tile([P, NT], F32, tag="rsq", bufs=2)
                nc.vector.scalar_tensor_tensor(
                    out=rsq, in0=t1, scalar=1.5, in1=y0, op0=ALU.add, op1=ALU.mult
                )
                return rsq

            def stage_c(b, expz, xb, mid_cb, last):
                """Y matmuls on unnormalized exp; one fp32 PSUM-drain per
                tile (p = y*xb -> fp16); sumsq on ACT; rsqrt + final scale
                on DVE; store via the gpsimd queue."""
                p_t = epil.tile([P, NT, S], F16, tag="p", bufs=2)
                ss = small.tile([P, NT], F32, tag="ss", bufs=2)
                outp = epil.tile([P, NT, S], F16, tag="outp", bufs=2)
                for tt in range(NT):
                    y = psY.tile([P, S], F32, tag="y")
                    for ut in range(NT):
                        nc.tensor.matmul(
                            y,
                            wv_sb[ut][:, tt * P : (tt + 1) * P],
                            expz[:, ut, :],
                            start=(ut == 0),
                            stop=(ut == NT - 1),
                        )
                    nc.vector.tensor_tensor(
                        out=p_t[:, tt, :], in0=y, in1=xb[:, tt, :], op=ALU.mult
                    )
                    scr = epil.tile([P, S], F16, tag="scr", bufs=2)
                    if last and tt % 2 == 1:
                        nc.vector.scalar_tensor_tensor(
                            out=scr,
                            in0=p_t[:, tt, :],
                            scalar=1.0,
                            in1=p_t[:, tt, :],
                            op0=ALU.mult,
                            op1=ALU.mult,
                            accum_out=ss[:, tt : tt + 1],
                        )
                    else:
                        nc.scalar.activation(
                            out=scr,
                            in_=p_t[:, tt, :],
                            func=AF.Square,
                            accum_out=ss[:, tt : tt + 1],
                        )
                    if tt == 1 and mid_cb is not None:
                        mid_cb()
                rsq = newton_rsqrt(ss)
                for tt in range(NT):
                    nc.vector.tensor_scalar(
                        out=outp[:, tt, :],
                        in0=p_t[:, tt, :],
                        scalar1=rsq[:, tt : tt + 1],
                        scalar2=None,
                        op0=ALU.mult,
                    )
                    (nc.sync if tt % 2 == 0 else nc.gpsimd).dma_start(
                        out=out_r[b, tt], in_=outp[:, tt, :]
                    )

            # ---- main software-pipelined batch loop (1-batch skew)
            pending = None  # (b, expz, esum, x1_sb) awaiting rowsum + stage C
            x1_cur, x2_cur = x1_first, x2_first
            for i in range(bpc):
                if i + 1 < bpc:
                    x1_nxt = dma_x1(i + 1)
                    x2_nxt = dma_x2(i + 1)
                else:
                    x1_nxt = x2_nxt = None
                xb_prev = (
                    stage_r(pending[2], pending[3]) if pending is not None else None
                )
                expz, esum = stage_a(x1_cur, x2_cur)
                if pending is not None:
                    stage_c(
                        pending[0],
                        pending[1],
                        xb_prev,
                        lambda e=expz, s=esum: finish_esum(e, s),
                        False,
                    )
                else:
                    finish_esum(expz, esum)
                pending = (i, expz, esum, x1_cur)
                x1_cur, x2_cur = x1_nxt, x2_nxt
            # drain: last batch
            xb_last = stage_r(pending[2], pending[3])
            stage_c(pending[0], pending[1], xb_last, None, True)

    nc.compile()
    return nc


_nc_cache = None


def kernel(x1, x2, W_query, W_key, W_value, _trace=False):
    global _nc_cache, last_results
    x1 = np.ascontiguousarray(
        np.asarray(x1, dtype=np.float32).transpose(0, 2, 1).astype(np.float16)
    )
    x2 = np.ascontiguousarray(
        np.asarray(x2, dtype=np.float32).transpose(0, 2, 1).astype(np.float16)
    )
    wq = np.asarray(W_query, dtype=np.float16)
    wk = np.asarray(W_key, dtype=np.float16)
    wv = np.asarray(W_value, dtype=np.float16)

    if _nc_cache is None:
        _nc_cache = build_nc()
    nc = _nc_cache

    in_maps = []
    for c in range(N_CORES):
        sl = slice(c * BPC, (c + 1) * BPC)
        in_maps.append(
            {"x1t": x1[sl], "x2t": x2[sl], "wq": wq, "wk": wk, "wv": wv}
        )
    res = run_bass_kernel_spmd(
        nc, in_maps, core_ids=list(range(N_CORES)), trace=_trace
    )
    last_results = res
    outT = np.concatenate([res.results[c]["out"] for c in range(N_CORES)], axis=0)
    return np.ascontiguousarray(outT.transpose(0, 2, 1).astype(np.float32))


# revision 11
# speedup vs baseline: 1.3633x; 1.3633x over previous
"""Trainium2 Bass kernel for nn_AttentionLayer (B=64, S=512, F=256), 8 cores.

Reference computation (per batch b):
    scores = x1 @ Wq + x2 @ Wk           # [S, S]
    a = softmax(tanh(scores), axis=-1)   # softmax over u
    a2 = a @ Wv                          # [S, S]
    out = a2 * x1                        # elementwise
    out = out * rsqrt(max(sum_s out^2, eps))   # l2-normalize over axis s

Strategy: pure data parallelism — 8 batches per core, weights replicated.
All tensors live in a TRANSPOSED layout ([t-or-u partitions, s free]) and in
fp16 end-to-end: same PE matmul rate as fp32r/bf16, half the HBM traffic of
fp32, and 2-4x DVE throughput vs fp32.

Engine budget per batch (errata-adjusted cost model, FD=512):
    PE   41 matmul passes (24 scores + 1 rowsum + 16 attn-out)   ~8.9us
    ACT  tanh+exp (in-place in PSUM) + 4x Square-with-accum      ~7.6us
    DVE  esum adds, recip, xb=x1*r, p=y*xb (PSUM drain), Newton
         rsqrt, final scales                                      ~7.2us
    Pool partition_broadcast + out-DMA issue                      ~2us

Key structural choices:
  * Softmax normalization DEFERRED past the second matmul:
    (exp(z)/D) @ Wv == (exp(z) @ Wv) / D (D varies only along the free
    axis s), so the TensorEngine never waits on the denominator chain.
    The 1/D scale is folded into x1 (xb = x1*r, off the critical path),
    leaving ONE fp32 PSUM-read op per output tile: p = y * xb.
  * The denominator's partition reduction: 4 exp u-tiles pre-summed on the
    DVE (fp16 adds), then a single ones-vector matmul pass on the PE.
  * l2-norm rsqrt on the DVE via Quake-style bit-trick seed + one Newton
    step (max rel err 0.18%) — the ACT engine only ever uses the
    exp_and_others table (tanh/exp/square): zero table reloads.
  * 1-batch software-pipeline skew; PE order [rowsum(b-1)][A(b)][C(b-1)].
"""

import sys

sys.path.insert(0, "/opt/trn_rl_repo")

import numpy as np

import concourse.bass as bass
import concourse.tile as tile
from concourse import bacc, mybir
from concourse.bass_utils import run_bass_kernel_spmd

B, S, F = 64, 512, 256
N_CORES = 8
BPC = B // N_CORES  # batches per core
P = 128
KT1 = S // P  # 4 k-tiles over t (x1/Wq contraction)
KT2 = F // P  # 2 k-tiles over f (x2/Wk contraction)
NT = S // P  # 4 m-tiles over u (stage A) / t (stage C)

F32 = mybir.dt.float32
F16 = mybir.dt.float16
I32 = mybir.dt.int32
AF = mybir.ActivationFunctionType
ALU = mybir.AluOpType

MAGIC = 0x5F375A86  # rsqrt seed constant

last_results = None  # test harness introspection


def build_nc(bpc=BPC):
    nc = bacc.Bacc(
        "TRN2", target_bir_lowering=False, debug=False, num_devices=N_CORES
    )
    x1t = nc.declare_dram_parameter("x1t", [bpc, S, S], F16, isOutput=False)
    x2t = nc.declare_dram_parameter("x2t", [bpc, F, S], F16, isOutput=False)
    wq = nc.declare_dram_parameter("wq", [S, S], F16, isOutput=False)
    wk = nc.declare_dram_parameter("wk", [F, S], F16, isOutput=False)
    wv = nc.declare_dram_parameter("wv", [S, S], F16, isOutput=False)
    out = nc.declare_dram_parameter("out", [bpc, S, S], F16, isOutput=True)

    x1t_r = x1t.ap().rearrange("b (a p) s -> b a p s", p=P)
    x2t_r = x2t.ap().rearrange("b (a p) s -> b a p s", p=P)
    out_r = out.ap().rearrange("b (a p) s -> b a p s", p=P)
    wq_r = wq.ap().rearrange("(a p) u -> a p u", p=P)
    wk_r = wk.ap().rearrange("(a p) u -> a p u", p=P)
    wv_r = wv.ap().rearrange("(a p) t -> a p t", p=P)

    with tile.TileContext(nc) as tc:
        with (
            tc.tile_pool(name="singles", bufs=1) as singles,
            tc.tile_pool(name="xin", bufs=1) as xin,
            tc.tile_pool(name="work", bufs=1) as work,
            tc.tile_pool(name="epil", bufs=1) as epil,
            tc.tile_pool(name="small", bufs=2) as small,
            tc.tile_pool(name="psA", bufs=2, space="PSUM") as psA,
            tc.tile_pool(name="psY", bufs=3, space="PSUM") as psY,
            tc.tile_pool(name="psR", bufs=1, space="PSUM") as psR,
        ):
            # ---- startup DMAs in first-use order, striped across the three
            # DMA-capable queues (scalar starts earliest on this runtime).
            wq_sb = [
                singles.tile([P, S], F16, tag=f"wq{kt}", name=f"wq{kt}")
                for kt in range(KT1)
            ]
            wk_sb = [
                singles.tile([P, S], F16, tag=f"wk{kt}", name=f"wk{kt}")
                for kt in range(KT2)
            ]
            wv_sb = [
                singles.tile([P, S], F16, tag=f"wv{ut}", name=f"wv{ut}")
                for ut in range(NT)
            ]
            x1_first = xin.tile([P, KT1, S], F16, tag="x1", bufs=4)
            x2_first = xin.tile([P, KT2, S], F16, tag="x2", bufs=3)
            nc.scalar.dma_start(out=wq_sb[0], in_=wq_r[0])
            nc.sync.dma_start(out=x1_first[:, 0, :], in_=x1t_r[0, 0])
            nc.gpsimd.dma_start(out=wq_sb[1], in_=wq_r[1])
            nc.scalar.dma_start(out=x1_first[:, 1, :], in_=x1t_r[0, 1])
            nc.sync.dma_start(out=wq_sb[2], in_=wq_r[2])
            nc.gpsimd.dma_start(out=x1_first[:, 2, :], in_=x1t_r[0, 2])
            nc.scalar.dma_start(out=wq_sb[3], in_=wq_r[3])
            nc.sync.dma_start(out=x1_first[:, 3, :], in_=x1t_r[0, 3])
            nc.gpsimd.dma_start(out=wk_sb[0], in_=wk_r[0])
            nc.scalar.dma_start(out=x2_first[:, 0, :], in_=x2t_r[0, 0])
            nc.sync.dma_start(out=wk_sb[1], in_=wk_r[1])
            nc.gpsimd.dma_start(out=x2_first[:, 1, :], in_=x2t_r[0, 1])
            for ut in range(NT):
                [nc.scalar, nc.sync, nc.gpsimd, nc.scalar][ut].dma_start(
                    out=wv_sb[ut], in_=wv_r[ut]
                )

            ones_col = singles.tile([P, 1], F16)
            nc.vector.memset(ones_col, 1.0)
            magic_t = singles.tile([P, NT], I32)
            nc.vector.memset(magic_t, MAGIC)
            shift1 = singles.tile([P, 1], I32)
            nc.vector.memset(shift1, 1)

            def dma_x1(b):
                t = xin.tile([P, KT1, S], F16, tag="x1", bufs=4)
                nc.sync.dma_start(out=t, in_=x1t_r[b].rearrange("a p s -> p a s"))
                return t

            def dma_x2(b):
                t = xin.tile([P, KT2, S], F16, tag="x2", bufs=3)
                nc.sync.dma_start(out=t, in_=x2t_r[b].rearrange("a p s -> p a s"))
                return t

            def stage_a(x1_sb, x2_sb):
                """scores matmuls (u-tile pairs sharing a 2-bank PSUM tile),
                tanh in-place in PSUM + exp on ACT, first exp pre-sum on DVE.
                Returns (expz, esum); esum still needs adds 2 and 3 (emitted
                later via finish_esum so mid-pipeline DVE work isn't blocked
                behind the exp of this batch)."""
                expz = work.tile([P, NT, S], F16, tag="expz", bufs=3)
                esum = work.tile([P, S], F16, tag="esum", bufs=3)
                for half in range(NT // 2):
                    sc = psA.tile([P, 2, S], F32, tag="scores")
                    for j in range(2):
                        ut = half * 2 + j
                        for kt in range(KT1):
                            nc.tensor.matmul(
                                sc[:, j, :],
                                wq_sb[kt][:, ut * P : (ut + 1) * P],
                                x1_sb[:, kt, :],
                                start=(kt == 0),
                                stop=False,
                            )
                        for kt in range(KT2):
                            nc.tensor.matmul(
                                sc[:, j, :],
                                wk_sb[kt][:, ut * P : (ut + 1) * P],
                                x2_sb[:, kt, :],
                                start=False,
                                stop=(kt == KT2 - 1),
                            )
                    nc.scalar.activation(out=sc, in_=sc, func=AF.Tanh)
                    nc.scalar.activation(
                        out=expz[:, half * 2 : half * 2 + 2, :],
                        in_=sc,
                        func=AF.Exp,
                    )
                return expz, esum

            def finish_esum(expz, esum):
                e01 = work.tile([P, 2, S], F16, tag="e01", bufs=2)
                nc.vector.tensor_tensor(
                    out=e01, in0=expz[:, 0:2, :], in1=expz[:, 2:4, :], op=ALU.add
                )
                nc.vector.tensor_tensor(
                    out=esum, in0=e01[:, 0, :], in1=e01[:, 1, :], op=ALU.add
                )

            def stage_r(esum, x1_sb):
                """deferred softmax denominator: one ones-matmul pass, fast
                reciprocal, fp16 cast, partition broadcast, then fold into
                x1: xb = x1 * (1/D) — all off the PE critical path."""
                rs = psR.tile([1, S], F32, tag="rowsum")
                nc.tensor.matmul(rs, ones_col, esum, start=True, stop=True)
                recip_f = small.tile([1, S], F32, tag="recipf")
                nc.vector.reciprocal_approx_fast(out=recip_f, in_=rs)
                recip_h = small.tile([1, S], F16, tag="reciph")
                nc.vector.tensor_copy(out=recip_h, in_=recip_f)
                bc = small.tile([P, S], F16, tag="bc")
                nc.gpsimd.partition_broadcast(bc, recip_h)
                xb = epil.tile([P, NT, S], F16, tag="xb", bufs=2)
                for tt in range(NT):
                    nc.vector.tensor_tensor(
                        out=xb[:, tt, :], in0=x1_sb[:, tt, :], in1=bc,
                        op=ALU.mult,
                    )
                return xb

            def newton_rsqrt(ss):
                """rsq ~ 1/sqrt(ss) on DVE via bit-trick seed + 1 Newton;
                result in fp16 so the final scales hit the 4x DVE mode."""
                ihalf = small.tile([P, NT], I32, tag="ihalf", bufs=2)
                nc.vector.tensor_scalar(
                    out=ihalf, in0=ss.bitcast(I32), scalar1=shift1, scalar2=None,
                    op0=ALU.logical_shift_right,
                )
                y0i = small.tile([P, NT], I32, tag="y0i", bufs=2)
                nc.vector.tensor_tensor(
                    out=y0i, in0=magic_t, in1=ihalf, op=ALU.subtract
                )
                y0 = y0i.bitcast(F32)
                t0 = small.tile([P, NT], F32, tag="nt0", bufs=2)
                nc.vector.tensor_tensor(out=t0, in0=y0, in1=y0, op=ALU.mult)
                t1 = small.tile([P, NT], F32, tag="nt1", bufs=2)
                nc.vector.scalar_tensor_tensor(
                    out=t1, in0=ss, scalar=-0.5, in1=t0, op0=ALU.mult, op1=ALU.mult
                )
                rsq = small.tile([P, NT], F32, tag="rsq", bufs=2)
                nc.vector.scalar_tensor_tensor(
                    out=rsq, in0=t1, scalar=1.5, in1=y0, op0=ALU.add, op1=ALU.mult
                )
                return rsq

            def stage_c(b, expz, xb, mid_cb, last):
                """Y matmuls on unnormalized exp; one fp32 PSUM-drain per
                tile (p = y*xb -> fp16); sumsq on ACT; rsqrt + final scale
                on DVE; store via the gpsimd queue."""
                p_t = epil.tile([P, NT, S], F16, tag="p", bufs=2)
                ss = small.tile([P, NT], F32, tag="ss", bufs=2)
                outp = epil.tile([P, NT, S], F16, tag="outp", bufs=2)
                for tt in range(NT):
                    y = psY.tile([P, S], F32, tag="y")
                    for ut in range(NT):
                        nc.tensor.matmul(
                            y,
                            wv_sb[ut][:, tt * P : (tt + 1) * P],
                            expz[:, ut, :],
                            start=(ut == 0),
                            stop=(ut == NT - 1),
                        )
                    nc.vector.tensor_tensor(
                        out=p_t[:, tt, :], in0=y, in1=xb[:, tt, :], op=ALU.mult
                    )
                    scr = epil.tile([P, S], F16, tag="scr", bufs=2)
                    if last and tt % 2 == 1:
                        nc.vector.scalar_tensor_tensor(
                            out=scr,
                            in0=p_t[:, tt, :],
                            scalar=1.0,
                            in1=p_t[:, tt, :],
                            op0=ALU.mult,
                            op1=ALU.mult,
                            accum_out=ss[:, tt : tt + 1],
                        )
                    else:
                        nc.scalar.activation(
                            out=scr,
                            in_=p_t[:, tt, :],
                            func=AF.Square,
                            accum_out=ss[:, tt : tt + 1],
                        )
                    if tt == 1 and mid_cb is not None:
                        mid_cb()
                rsq = newton_rsqrt(ss)
                for tt in range(NT):
                    nc.vector.tensor_scalar(
                        out=outp[:, tt, :],
                        in0=p_t[:, tt, :],
                        scalar1=rsq[:, tt : tt + 1],
                        scalar2=None,
                        op0=ALU.mult,
                    )
                    (nc.sync if tt % 2 == 0 else nc.gpsimd).dma_start(
                        out=out_r[b, tt], in_=outp[:, tt, :]
                    )

            # ---- main software-pipelined batch loop (1-batch skew)
            pending = None  # (b, expz, esum, x1_sb) awaiting rowsum + stage C
            x1_cur, x2_cur = x1_first, x2_first
            for i in range(bpc):
                if i + 1 < bpc:
                    x1_nxt = dma_x1(i + 1)
                    x2_nxt = dma_x2(i + 1)
                else:
                    x1_nxt = x2_nxt = None
                xb_prev = (
                    stage_r(pending[2], pending[3]) if pending is not None else None
                )
                expz, esum = stage_a(x1_cur, x2_cur)
                if pending is not None:
                    stage_c(
                        pending[0],
                        pending[1],
                        xb_prev,
                        lambda e=expz, s=esum: finish_esum(e, s),
                        False,
                    )
                else:
                    finish_esum(expz, esum)
                pending = (i, expz, esum, x1_cur)
                x1_cur, x2_cur = x1_nxt, x2_nxt
            # drain: last batch
            xb_last = stage_r(pending[2], pending[3])
            stage_c(pending[0], pending[1], xb_last, None, True)

    nc.compile()
    return nc


_nc_cache = None


def kernel(x1, x2, W_query, W_key, W_value, _trace=False):
    global _nc_cache, last_results
    x1 = np.ascontiguousarray(
        np.asarray(x1, dtype=np.float32).transpose(0, 2, 1).astype(np.float16)
    )
    x2 = np.ascontiguousarray(
        np.asarray(x2, dtype=np.float32).transpose(0, 2, 1).astype(np.float16)
    )
    wq = np.asarray(W_query, dtype=np.float16)
    wk = np.asarray(W_key, dtype=np.float16)
    wv = np.asarray(W_value, dtype=np.float16)

    if _nc_cache is None:
        _nc_cache = build_nc()
    nc = _nc_cache

    in_maps = []
    for c in range(N_CORES):
        sl = slice(c * BPC, (c + 1) * BPC)
        in_maps.append(
            {"x1t": x1[sl], "x2t": x2[sl], "wq": wq, "wk": wk, "wv": wv}
        )
    res = run_bass_kernel_spmd(
        nc, in_maps, core_ids=list(range(N_CORES)), trace=_trace
    )
    last_results = res
    outT = np.concatenate([res.results[c]["out"] for c in range(N_CORES)], axis=0)
    return np.ascontiguousarray(outT.transpose(0, 2, 1).astype(np.float32))


# revision 12
# speedup vs baseline: 1.3824x; 1.0141x over previous
"""Trainium2 Bass kernel for nn_AttentionLayer (B=64, S=512, F=256), 8 cores.

Reference computation (per batch b):
    scores = x1 @ Wq + x2 @ Wk           # [S, S]
    a = softmax(tanh(scores), axis=-1)   # softmax over u
    a2 = a @ Wv                          # [S, S]
    out = a2 * x1                        # elementwise
    out = out * rsqrt(max(sum_s out^2, eps))   # l2-normalize over axis s

Strategy: pure data parallelism — 8 batches per core, weights replicated.
All tensors live in a TRANSPOSED layout ([t-or-u partitions, s free]) and in
fp16 end-to-end: same PE matmul rate as fp32r/bf16, half the HBM traffic of
fp32, and 2-4x DVE throughput vs fp32.

Engine budget per batch (errata-adjusted cost model, FD=512):
    PE   41 matmul passes (24 scores + 1 rowsum + 16 attn-out)   ~8.9us
    ACT  tanh+exp (in-place in PSUM) + 4x Square-with-accum      ~7.6us
    DVE  esum adds, recip, xb=x1*r, p=y*xb (PSUM drain), Newton
         rsqrt, final scales                                      ~7.2us
    Pool partition_broadcast + out-DMA issue                      ~2us

Key structural choices:
  * Softmax normalization DEFERRED past the second matmul:
    (exp(z)/D) @ Wv == (exp(z) @ Wv) / D (D varies only along the free
    axis s), so the TensorEngine never waits on the denominator chain.
    The 1/D scale is folded into x1 (xb = x1*r, off the critical path),
    leaving ONE fp32 PSUM-read op per output tile: p = y * xb.
  * The denominator's partition reduction: 4 exp u-tiles pre-summed on the
    DVE (fp16 adds), then a single ones-vector matmul pass on the PE.
  * l2-norm rsqrt on the DVE via Quake-style bit-trick seed + one Newton
    step (max rel err 0.18%) — the ACT engine only ever uses the
    exp_and_others table (tanh/exp/square): zero table reloads.
  * 1-batch software-pipeline skew; PE order [rowsum(b-1)][A(b)][C(b-1)].
"""

import sys

sys.path.insert(0, "/opt/trn_rl_repo")

import numpy as np

import concourse.bass as bass
import concourse.tile as tile
from concourse import bacc, mybir
from concourse.bass_utils import run_bass_kernel_spmd

B, S, F = 64, 512, 256
N_CORES = 8
BPC = B // N_CORES  # batches per core
P = 128
KT1 = S // P  # 4 k-tiles over t (x1/Wq contraction)
KT2 = F // P  # 2 k-tiles over f (x2/Wk contraction)
NT = S // P  # 4 m-tiles over u (stage A) / t (stage C)

F32 = mybir.dt.float32
F16 = mybir.dt.float16
I32 = mybir.dt.int32
AF = mybir.ActivationFunctionType
ALU = mybir.AluOpType

MAGIC = 0x5F375A86  # rsqrt seed constant

last_results = None  # test harness introspection


def build_nc(bpc=BPC):
    nc = bacc.Bacc(
        "TRN2", target_bir_lowering=False, debug=False, num_devices=N_CORES
    )
    x1t = nc.declare_dram_parameter("x1t", [bpc, S, S], F16, isOutput=False)
    x2t = nc.declare_dram_parameter("x2t", [bpc, F, S], F16, isOutput=False)
    wq = nc.declare_dram_parameter("wq", [S, S], F16, isOutput=False)
    wk = nc.declare_dram_parameter("wk", [F, S], F16, isOutput=False)
    wv = nc.declare_dram_parameter("wv", [S, S], F16, isOutput=False)
    out = nc.declare_dram_parameter("out", [bpc, S, S], F16, isOutput=True)

    x1t_r = x1t.ap().rearrange("b (a p) s -> b a p s", p=P)
    x2t_r = x2t.ap().rearrange("b (a p) s -> b a p s", p=P)
    out_r = out.ap().rearrange("b (a p) s -> b a p s", p=P)
    wq_r = wq.ap().rearrange("(a p) u -> a p u", p=P)
    wk_r = wk.ap().rearrange("(a p) u -> a p u", p=P)
    wv_r = wv.ap().rearrange("(a p) t -> a p t", p=P)

    with tile.TileContext(nc) as tc:
        with (
            tc.tile_pool(name="singles", bufs=1) as singles,
            tc.tile_pool(name="xin", bufs=1) as xin,
            tc.tile_pool(name="work", bufs=1) as work,
            tc.tile_pool(name="epil", bufs=1) as epil,
            tc.tile_pool(name="small", bufs=2) as small,
            tc.tile_pool(name="psA", bufs=2, space="PSUM") as psA,
            tc.tile_pool(name="psY", bufs=3, space="PSUM") as psY,
            tc.tile_pool(name="psR", bufs=1, space="PSUM") as psR,
        ):
            # ---- startup DMAs in first-use order, striped across the three
            # DMA-capable queues (scalar starts earliest on this runtime).
            wq_sb = [
                singles.tile([P, S], F16, tag=f"wq{kt}", name=f"wq{kt}")
                for kt in range(KT1)
            ]
            wk_sb = [
                singles.tile([P, S], F16, tag=f"wk{kt}", name=f"wk{kt}")
                for kt in range(KT2)
            ]
            wv_sb = [
                singles.tile([P, S], F16, tag=f"wv{ut}", name=f"wv{ut}")
                for ut in range(NT)
            ]
            x1_first = xin.tile([P, KT1, S], F16, tag="x1", bufs=4)
            x2_first = xin.tile([P, KT2, S], F16, tag="x2", bufs=3)
            nc.scalar.dma_start(out=wq_sb[0], in_=wq_r[0])
            nc.sync.dma_start(out=x1_first[:, 0, :], in_=x1t_r[0, 0])
            nc.gpsimd.dma_start(out=wq_sb[1], in_=wq_r[1])
            nc.scalar.dma_start(out=x1_first[:, 1, :], in_=x1t_r[0, 1])
            nc.sync.dma_start(out=wq_sb[2], in_=wq_r[2])
            nc.gpsimd.dma_start(out=x1_first[:, 2, :], in_=x1t_r[0, 2])
            nc.scalar.dma_start(out=wq_sb[3], in_=wq_r[3])
            nc.sync.dma_start(out=x1_first[:, 3, :], in_=x1t_r[0, 3])
            nc.gpsimd.dma_start(out=wk_sb[0], in_=wk_r[0])
            nc.scalar.dma_start(out=x2_first[:, 0, :], in_=x2t_r[0, 0])
            nc.sync.dma_start(out=wk_sb[1], in_=wk_r[1])
            nc.gpsimd.dma_start(out=x2_first[:, 1, :], in_=x2t_r[0, 1])
            for ut in range(NT):
                [nc.scalar, nc.sync, nc.gpsimd, nc.scalar][ut].dma_start(
                    out=wv_sb[ut], in_=wv_r[ut]
                )

            ones_col = singles.tile([P, 1], F16)
            nc.vector.memset(ones_col, 1.0)
            magic_t = singles.tile([P, NT], I32)
            nc.vector.memset(magic_t, MAGIC)
            shift1 = singles.tile([P, 1], I32)
            nc.vector.memset(shift1, 1)

            def dma_x1(b):
                t = xin.tile([P, KT1, S], F16, tag="x1", bufs=4)
                nc.sync.dma_start(out=t, in_=x1t_r[b].rearrange("a p s -> p a s"))
                return t

            def dma_x2(b):
                t = xin.tile([P, KT2, S], F16, tag="x2", bufs=3)
                nc.sync.dma_start(out=t, in_=x2t_r[b].rearrange("a p s -> p a s"))
                return t

            def stage_a(x1_sb, x2_sb):
                """scores matmuls (u-tile pairs sharing a 2-bank PSUM tile),
                tanh in-place in PSUM + exp on ACT, first exp pre-sum on DVE.
                Returns (expz, esum); esum still needs adds 2 and 3 (emitted
                later via finish_esum so mid-pipeline DVE work isn't blocked
                behind the exp of this batch)."""
                expz = work.tile([P, NT, S], F16, tag="expz", bufs=3)
                esum = work.tile([P, S], F16, tag="esum", bufs=3)
                for half in range(NT // 2):
                    sc = psA.tile([P, 2, S], F32, tag="scores")
                    for j in range(2):
                        ut = half * 2 + j
                        for kt in range(KT1):
                            nc.tensor.matmul(
                                sc[:, j, :],
                                wq_sb[kt][:, ut * P : (ut + 1) * P],
                                x1_sb[:, kt, :],
                                start=(kt == 0),
                                stop=False,
                            )
                        for kt in range(KT2):
                            nc.tensor.matmul(
                                sc[:, j, :],
                                wk_sb[kt][:, ut * P : (ut + 1) * P],
                                x2_sb[:, kt, :],
                                start=False,
                                stop=(kt == KT2 - 1),
                            )
                    nc.scalar.activation(out=sc, in_=sc, func=AF.Tanh)
                    nc.scalar.activation(
                        out=expz[:, half * 2 : half * 2 + 2, :],
                        in_=sc,
                        func=AF.Exp,
                    )
                return expz, esum

            def finish_esum(expz, esum):
                e01 = work.tile([P, 2, S], F16, tag="e01", bufs=2)
                nc.vector.tensor_tensor(
                    out=e01, in0=expz[:, 0:2, :], in1=expz[:, 2:4, :], op=ALU.add
                )
                nc.vector.tensor_tensor(
                    out=esum, in0=e01[:, 0, :], in1=e01[:, 1, :], op=ALU.add
                )

            def stage_r(esum, x1_sb, lazy=False):
                """deferred softmax denominator: one ones-matmul pass, fast
                reciprocal, fp16 cast, partition broadcast, then fold into
                x1: xb = x1 * (1/D) — all off the PE critical path."""
                rs = psR.tile([1, S], F32, tag="rowsum")
                nc.tensor.matmul(rs, ones_col, esum, start=True, stop=True)
                recip_f = small.tile([1, S], F32, tag="recipf")
                nc.vector.reciprocal_approx_fast(out=recip_f, in_=rs)
                recip_h = small.tile([1, S], F16, tag="reciph")
                nc.vector.tensor_copy(out=recip_h, in_=recip_f)
                bc = small.tile([P, S], F16, tag="bc")
                nc.gpsimd.partition_broadcast(bc, recip_h)
                xb = epil.tile([P, NT, S], F16, tag="xb", bufs=2)

                def emit_xb(tt):
                    nc.vector.tensor_tensor(
                        out=xb[:, tt, :], in0=x1_sb[:, tt, :], in1=bc,
                        op=ALU.mult,
                    )
                    return xb[:, tt, :]

                if lazy:
                    return emit_xb
                for tt in range(NT):
                    emit_xb(tt)
                return xb

            def newton_rsqrt(ss):
                """rsq ~ 1/sqrt(ss) on DVE via bit-trick seed + 1 Newton;
                result in fp16 so the final scales hit the 4x DVE mode."""
                ihalf = small.tile([P, NT], I32, tag="ihalf", bufs=2)
                nc.vector.tensor_scalar(
                    out=ihalf, in0=ss.bitcast(I32), scalar1=shift1, scalar2=None,
                    op0=ALU.logical_shift_right,
                )
                y0i = small.tile([P, NT], I32, tag="y0i", bufs=2)
                nc.vector.tensor_tensor(
                    out=y0i, in0=magic_t, in1=ihalf, op=ALU.subtract
                )
                y0 = y0i.bitcast(F32)
                t0 = small.tile([P, NT], F32, tag="nt0", bufs=2)
                nc.vector.tensor_tensor(out=t0, in0=y0, in1=y0, op=ALU.mult)
                t1 = small.tile([P, NT], F32, tag="nt1", bufs=2)
                nc.vector.scalar_tensor_tensor(
                    out=t1, in0=ss, scalar=-0.5, in1=t0, op0=ALU.mult, op1=ALU.mult
                )
                rsq = small.tile([P, NT], F32, tag="rsq", bufs=2)
                nc.vector.scalar_tensor_tensor(
                    out=rsq, in0=t1, scalar=1.5, in1=y0, op0=ALU.add, op1=ALU.mult
                )
                return rsq

            def stage_c(b, expz, xb, mid_cb=None, tt_cb=None, fin=True):
                """Y matmuls on unnormalized exp; one fp32 PSUM-drain per
                tile (p = y*xb -> fp16, xb possibly emitted lazily per tile);
                sumsq on ACT; rsqrt + final scale on DVE; store split across
                the sync/gpsimd DMA queues.  With fin=False the
                newton+scale+store part is returned as (newton_fn, scale_fn)
                so the drain can interleave it between the LAST batch's
                PSUM-drains (keeps psY recycling fast at the pipeline end)."""
                p_t = epil.tile([P, NT, S], F16, tag="p", bufs=2)
                ss = small.tile([P, NT], F32, tag="ss", bufs=2)
                outp = epil.tile([P, NT, S], F16, tag="outp", bufs=2)
                for tt in range(NT):
                    y = psY.tile([P, S], F32, tag="y")
                    for ut in range(NT):
                        nc.tensor.matmul(
                            y,
                            wv_sb[ut][:, tt * P : (tt + 1) * P],
                            expz[:, ut, :],
                            start=(ut == 0),
                            stop=(ut == NT - 1),
                        )
                    xb_t = xb(tt) if callable(xb) else xb[:, tt, :]
                    nc.vector.tensor_tensor(
                        out=p_t[:, tt, :], in0=y, in1=xb_t, op=ALU.mult
                    )
                    scr = epil.tile([P, S], F16, tag="scr", bufs=2)
                    nc.scalar.activation(
                        out=scr,
                        in_=p_t[:, tt, :],
                        func=AF.Square,
                        accum_out=ss[:, tt : tt + 1],
                    )
                    if tt_cb is not None:
                        tt_cb(tt)
                    if tt == 1 and mid_cb is not None:
                        mid_cb()

                def newton_fn():
                    return newton_rsqrt(ss)

                def scale_fn(rsq, tt):
                    nc.vector.tensor_scalar(
                        out=outp[:, tt, :],
                        in0=p_t[:, tt, :],
                        scalar1=rsq[:, tt : tt + 1],
                        scalar2=None,
                        op0=ALU.mult,
                    )
                    (nc.sync if tt % 2 == 0 else nc.gpsimd).dma_start(
                        out=out_r[b, tt], in_=outp[:, tt, :]
                    )

                if fin:
                    rsq = newton_fn()
                    for tt in range(NT):
                        scale_fn(rsq, tt)
                    return None
                return newton_fn, scale_fn

            # ---- main software-pipelined batch loop (1-batch skew)
            pending = None  # (b, expz, esum, x1_sb) awaiting rowsum + stage C
            deferred = None  # second-to-last batch's (newton_fn, scale_fn)
            x1_cur, x2_cur = x1_first, x2_first
            for i in range(bpc):
                if i + 1 < bpc:
                    x1_nxt = dma_x1(i + 1)
                    x2_nxt = dma_x2(i + 1)
                else:
                    x1_nxt = x2_nxt = None
                xb_prev = (
                    stage_r(pending[2], pending[3]) if pending is not None else None
                )
                expz, esum = stage_a(x1_cur, x2_cur)
                if pending is not None:
                    deferred = stage_c(
                        pending[0],
                        pending[1],
                        xb_prev,
                        mid_cb=lambda e=expz, s=esum: finish_esum(e, s),
                        fin=(i + 1 < bpc),
                    )
                else:
                    finish_esum(expz, esum)
                pending = (i, expz, esum, x1_cur)
                x1_cur, x2_cur = x1_nxt, x2_nxt
            # ---- drain.  The last batch's epilogue interleaves with the
            # deferred finalization of the second-to-last batch so the DVE
            # queue serves the psY drains first.
            xb_last = stage_r(pending[2], pending[3], lazy=True)
            if deferred is not None:
                d_newton, d_scale = deferred
                d_rsq = d_newton()
                tt_cb = lambda tt: d_scale(d_rsq, tt)
            else:
                tt_cb = None
            stage_c(pending[0], pending[1], xb_last, tt_cb=tt_cb, fin=True)

    nc.compile()
    return nc


_nc_cache = None


def kernel(x1, x2, W_query, W_key, W_value, _trace=False):
    global _nc_cache, last_results
    x1 = np.ascontiguousarray(
        np.asarray(x1, dtype=np.float32).transpose(0, 2, 1).astype(np.float16)
    )
    x2 = np.ascontiguousarray(
        np.asarray(x2, dtype=np.float32).transpose(0, 2, 1).astype(np.float16)
    )
    wq = np.asarray(W_query, dtype=np.float16)
    wk = np.asarray(W_key, dtype=np.float16)
    wv = np.asarray(W_value, dtype=np.float16)

    if _nc_cache is None:
        _nc_cache = build_nc()
    nc = _nc_cache

    in_maps = []
    for c in range(N_CORES):
        sl = slice(c * BPC, (c + 1) * BPC)
        in_maps.append(
            {"x1t": x1[sl], "x2t": x2[sl], "wq": wq, "wk": wk, "wv": wv}
        )
    res = run_bass_kernel_spmd(
        nc, in_maps, core_ids=list(range(N_CORES)), trace=_trace
    )
    last_results = res
    outT = np.concatenate([res.results[c]["out"] for c in range(N_CORES)], axis=0)
    return np.ascontiguousarray(outT.transpose(0, 2, 1).astype(np.float32))
